# revision 2
# baseline (speedup 1.0000x reference)
"""Bass/Trainium2 kernel v2 for nn_DecoderModel (B=4 T=1024 D=1024 H=16 L=12 V=50257).

Sharding: 8 cores; core c = (batch b=c//2, parity p=c%2). Parity p owns the
4 global 128-token q-tiles {2j+1-p}. Residual stream transposed in SBUF as
[128, 8, 512] (d-partition, d-tile, token).

v2 changes vs baseline:
- Weight panel DMAs (host pre-transposed layouts, 0.5-2MB contiguous per DMA)
  instead of ~700 small per-matmul DMAs per layer.
- Attention split into local pass (own K/V from SBUF, runs during the pair
  AllGather) + remote pass (peer block from the gathered buffer, selected with
  a partition-id-derived dynamic DMA offset).
- Softmax denominator via concurrent col-tiled ones-matmul; score matmuls of
  the two heads of a pair run concurrently via row tiling (K=64).
- LM head loads each lm_W panel once (4 PSUM banks accumulate all 4 q-tiles),
  bias applied by a K=1 ones matmul, bf16 output.
"""
import os
import sys

sys.path.insert(0, "/opt/trn_rl_repo")

import numpy as np
import ml_dtypes

import concourse.bass as bass
import concourse.mybir as mybir
import concourse.tile as tile
from concourse import bacc
from concourse.bass_utils import run_bass_kernel_spmd

BF16 = mybir.dt.bfloat16
F32 = mybir.dt.float32

B, T, D, H, NL_FULL, V = 4, 1024, 1024, 16, 12, 50257
DH = D // H              # 64
DT = D // 128            # 8 d-tiles
QT = 4                   # q-tiles (128 rows) per core
VPAD = 50688             # 99 * 512
NVG = VPAD // 512        # 99 vocab groups
LN_EPS = 1e-5
INV_SQRT_C = 1.0 / 32.0

L = int(os.environ.get("BASSK_L", str(NL_FULL)))

KT_ELEMS = 128 * DT * 512          # K^T block: [p, ft, tok]
V_ELEMS = 128 * H * 4 * 64         # V block: [p(tok), h, kslot, e]
AG_IN_ELEMS = KT_ELEMS + V_ELEMS
AG_OUT_ELEMS = 2 * AG_IN_ELEMS


def build_nc(num_layers=L):
    nc = bacc.Bacc("TRN2", target_bir_lowering=False, debug=True)
    NL = num_layers

    x0T = nc.declare_dram_parameter("x0T", [128, DT, 512], F32, isOutput=False)
    wqp = nc.declare_dram_parameter("wqp", [NL, 128, DT, DT, 128], BF16, isOutput=False)
    wkp = nc.declare_dram_parameter("wkp", [NL, 128, DT, DT, 128], BF16, isOutput=False)
    wvp = nc.declare_dram_parameter("wvp", [NL, 128, 2, DT, 512], BF16, isOutput=False)
    wop = nc.declare_dram_parameter("wop", [NL, 128, DT, DT, 128], BF16, isOutput=False)
    w1p = nc.declare_dram_parameter("w1p", [NL, 128, 32, DT, 128], BF16, isOutput=False)
    w2p = nc.declare_dram_parameter("w2p", [NL, 128, DT, 32, 128], BF16, isOutput=False)
    ln1s = nc.declare_dram_parameter("ln1s", [NL, 128, DT], F32, isOutput=False)
    ln1b = nc.declare_dram_parameter("ln1b", [NL, 128, DT], F32, isOutput=False)
    ln2s = nc.declare_dram_parameter("ln2s", [NL, 128, DT], F32, isOutput=False)
    ln2b = nc.declare_dram_parameter("ln2b", [NL, 128, DT], F32, isOutput=False)
    bo_p = nc.declare_dram_parameter("bo_p", [NL, 128, DT], F32, isOutput=False)
    b1_p = nc.declare_dram_parameter("b1_p", [NL, 128, 32], F32, isOutput=False)
    b2_p = nc.declare_dram_parameter("b2_p", [NL, 128, DT], F32, isOutput=False)
    lnfs = nc.declare_dram_parameter("lnfs", [128, DT], F32, isOutput=False)
    lnfb = nc.declare_dram_parameter("lnfb", [128, DT], F32, isOutput=False)
    lmwp = nc.declare_dram_parameter("lmwp", [NVG, 128, DT, 512], BF16, isOutput=False)
    lmb_r = nc.declare_dram_parameter("lmb_r", [NVG, 1, 512], BF16, isOutput=False)
    # masks[0] = tril (local diag block), masks[1] = remote diag block
    # (ones for parity 0, zeros for parity 1)
    masks = nc.declare_dram_parameter("masks", [2, 128, 128], BF16, isOutput=False)
    # peer block index (1 - parity) for dynamic ag_out addressing
    peer_i = nc.declare_dram_parameter("peer_i", [1, 1], mybir.dt.int32,
                                       isOutput=False)
    out = nc.declare_dram_parameter("out", [512, VPAD], BF16, isOutput=True)

    ag_in = [nc.dram_tensor(f"ag_in{i}", [AG_IN_ELEMS], BF16) for i in range(2)]
    ag_out = [nc.dram_tensor(f"ag_out{i}", [AG_OUT_ELEMS], BF16) for i in range(2)]
    groups = [[0, 1], [2, 3], [4, 5], [6, 7]]

    from contextlib import ExitStack
    with tile.TileContext(nc) as tc, ExitStack() as es:
        const = es.enter_context(tc.tile_pool(name="const", bufs=1))
        xpool = es.enter_context(tc.tile_pool(name="xpool", bufs=1))
        npool = es.enter_context(tc.tile_pool(name="npool", bufs=1))
        lntp = es.enter_context(tc.tile_pool(name="lntp", bufs=1))
        small = es.enter_context(tc.tile_pool(name="small", bufs=1))

        ones_bf = const.tile([128, 1], BF16)
        nc.vector.memset(ones_bf[:], 1.0)
        ones_bf1 = const.tile([1, 128], BF16)
        nc.vector.memset(ones_bf1[:], 1.0)
        onesf2 = const.tile([128, 128], F32)
        nc.vector.memset(onesf2[:], 1.0)
        eps_t = const.tile([1, 1], F32)
        nc.vector.memset(eps_t[:], LN_EPS)
        mask_t = const.tile([128, 2, 128], BF16)
        nc.sync.dma_start(mask_t[:], masks.rearrange("m k q -> k m q"))
        lnf_s_t = const.tile([128, DT], F32)
        nc.sync.dma_start(lnf_s_t[:], lnfs[:])
        lnf_b_t = const.tile([128, DT], F32)
        nc.sync.dma_start(lnf_b_t[:], lnfb[:])

        xT = xpool.tile([128, DT, 512], F32, name="xT", tag="x")
        nc.sync.dma_start(xT[:], x0T[:])

        # peer block index for the gathered buffer (register on gpsimd)
        peer_reg = nc.gpsimd.alloc_register("peer_reg")
        nc.gpsimd.reg_load(peer_reg, peer_i[0:1, 0:1])
        peer = nc.gpsimd.snap(peer_reg, donate=True, min_val=0, max_val=1)

        def layernorm(x_in, s_dram, b_dram):
            """x_in: [128, DT, 512] f32 -> nbf [128, DT, 512] bf16."""
            if s_dram is not None:
                s_t = small.tile([128, DT], F32, name="lns", tag="lns")
                nc.sync.dma_start(s_t[:], s_dram)
                b_t = small.tile([128, DT], F32, name="lnb", tag="lnb")
                nc.sync.dma_start(b_t[:], b_dram)
            else:
                s_t, b_t = lnf_s_t, lnf_b_t
            xbf = lntp.tile([128, DT, 512], BF16, name="xbf", tag="xbf")
            nc.vector.tensor_copy(xbf[:], x_in[:])
            sq = lntp.tile([128, DT, 512], BF16, name="sq", tag="sq")
            nc.scalar.square(sq[:], x_in[:])
            with tc.tile_pool(name="lnp", bufs=1, space="PSUM") as lnp:
                ps1 = lnp.tile([1, 512], F32, name="ps1", tag="ps1")
                ps2 = lnp.tile([1, 512], F32, name="ps2", tag="ps2")
                for dt_i in range(DT):
                    nc.tensor.matmul(ps1[:], ones_bf[:], xbf[:, dt_i],
                                     start=(dt_i == 0), stop=(dt_i == DT - 1))
                for dt_i in range(DT):
                    nc.tensor.matmul(ps2[:], ones_bf[:], sq[:, dt_i],
                                     start=(dt_i == 0), stop=(dt_i == DT - 1))
                mu = small.tile([1, 512], F32, name="mu", tag="mu")
                nc.vector.tensor_scalar_mul(mu[:], ps1[:], 1.0 / D)
                var = small.tile([1, 512], F32, name="var", tag="var")
                nc.vector.tensor_scalar_mul(var[:], ps2[:], 1.0 / D)
                musq = small.tile([1, 512], F32, name="musq", tag="musq")
                nc.vector.tensor_mul(musq[:], mu[:], mu[:])
                nc.vector.tensor_sub(var[:], var[:], musq[:])
                sd = small.tile([1, 512], F32, name="sd", tag="sd")
                nc.scalar.activation(sd[:], var[:],
                                     mybir.ActivationFunctionType.Sqrt,
                                     bias=eps_t[:])
                rstd = small.tile([1, 512], F32, name="rstd", tag="rstd")
                nc.vector.reciprocal(rstd[:], sd[:])
                mub = lnp.tile([128, 512], F32, name="mub", tag="mub")
                nc.tensor.matmul(mub[:], onesf2[0:1, :], mu[:], start=True,
                                 stop=True)
                rstdb = lnp.tile([128, 512], F32, name="rstdb", tag="rstdb")
                nc.tensor.matmul(rstdb[:], onesf2[0:1, :], rstd[:], start=True,
                                 stop=True)
                nbf = npool.tile([128, DT, 512], BF16, name="nbf", tag="nbf")
                for dt_i in range(DT):
                    t1 = small.tile([128, 512], F32, name="lnt1", tag="lnt1",
                                    bufs=2)
                    nc.vector.tensor_sub(t1[:], x_in[:, dt_i], mub[:])
                    t2 = small.tile([128, 512], F32, name="lnt2", tag="lnt2",
                                    bufs=2)
                    nc.vector.tensor_mul(t2[:], t1[:], rstdb[:])
                    nc.scalar.activation(nbf[:, dt_i], t2[:],
                                         mybir.ActivationFunctionType.Identity,
                                         bias=b_t[:, dt_i:dt_i + 1],
                                         scale=s_t[:, dt_i:dt_i + 1])
            return nbf

        with ExitStack() as les:
            proj = les.enter_context(tc.tile_pool(name="proj", bufs=1))
            wpool = les.enter_context(tc.tile_pool(name="wpool", bufs=2))
            locp = les.enter_context(tc.tile_pool(name="locp", bufs=1))
            stp = les.enter_context(tc.tile_pool(name="stp", bufs=2))
            sump = les.enter_context(tc.tile_pool(name="sump", bufs=1))
            rem = les.enter_context(tc.tile_pool(name="rem", bufs=3))

            for l in range(L):
                slot = l % 2
                n1 = layernorm(xT, ln1s[l], ln1b[l])

                # ---- K^T / V projections (panel weight loads)
                kt_sb = proj.tile([128, DT, 512], BF16, name="kt_sb", tag="big")
                v_sb = proj.tile([128, H, 4, 64], BF16, name="v_sb", tag="vh")
                with tc.tile_pool(name="pqkv", bufs=1, space="PSUM") as pqkv:
                    for half in range(2):
                        wk_t = wpool.tile([128, 4, DT, 128], BF16, name="wk_t",
                                          tag="wbig")
                        nc.sync.dma_start(wk_t[:],
                                          wkp[l, :, half * 4:(half + 1) * 4])
                        for fi in range(4):
                            ft = half * 4 + fi
                            pq = pqkv.tile([128, 512], F32, name="pq", tag="pq",
                                           bufs=3)
                            for dt_i in range(DT):
                                nc.tensor.matmul(pq[:], wk_t[:, fi, dt_i],
                                                 n1[:, dt_i],
                                                 start=(dt_i == 0),
                                                 stop=(dt_i == DT - 1))
                            nc.scalar.copy(kt_sb[:, ft], pq[:])
                    for half in range(2):
                        wv_t = wpool.tile([128, 1, DT, 512], BF16, name="wv_t",
                                          tag="wbig")
                        nc.sync.dma_start(wv_t[:], wvp[l, :, half:half + 1])
                        pvs = [pqkv.tile([128, 512], F32, name="pv", tag="pv",
                                         bufs=4) for _ in range(4)]
                        for dt_i in range(DT):
                            for tt in range(4):
                                nc.tensor.matmul(
                                    pvs[tt][:],
                                    n1[:, dt_i, tt * 128:(tt + 1) * 128],
                                    wv_t[:, 0, dt_i], start=(dt_i == 0),
                                    stop=(dt_i == DT - 1))
                        for tt in range(4):
                            nc.scalar.copy(
                                v_sb[:, half * 8:half * 8 + 8, tt, :],
                                pvs[tt].rearrange("p (h e) -> p h e", h=8))

                    # ---- ship K,V; start pair AllGather
                    nc.sync.dma_start(
                        ag_in[slot][0:KT_ELEMS].rearrange(
                            "(p a t) -> p a t", p=128, a=DT), kt_sb[:])
                    nc.sync.dma_start(
                        ag_in[slot][KT_ELEMS:].rearrange(
                            "(p y) -> p y", p=128),
                        v_sb.rearrange("p h k e -> p (h k e)"))
                    nc.gpsimd.collective_compute(
                        "AllGather", mybir.AluOpType.bypass,
                        replica_groups=groups,
                        ins=[ag_in[slot][:]], outs=[ag_out[slot][:]])

                    # ---- Q^T projection
                    qt_sb = proj.tile([128, DT, 512], BF16, name="qt_sb",
                                      tag="qt")
                    for half in range(2):
                        wq_t = wpool.tile([128, 4, DT, 128], BF16, name="wq_t",
                                          tag="wbig")
                        nc.sync.dma_start(wq_t[:],
                                          wqp[l, :, half * 4:(half + 1) * 4])
                        for fi in range(4):
                            ft = half * 4 + fi
                            pq = pqkv.tile([128, 512], F32, name="pq", tag="pq",
                                           bufs=3)
                            for dt_i in range(DT):
                                nc.tensor.matmul(pq[:], wq_t[:, fi, dt_i],
                                                 n1[:, dt_i],
                                                 start=(dt_i == 0),
                                                 stop=(dt_i == DT - 1))
                            nc.scalar.copy(qt_sb[:, ft], pq[:])

                # ---- attention
                locA = locp.tile([128, DT, 512], BF16, name="locA", tag="locA")
                locB = locp.tile([128, DT, 512], BF16, name="locB", tag="locB")
                oT_all = proj.tile([128, DT, 512], BF16, name="oT_all",
                                   tag="big")
                blk = ag_out[slot].rearrange("(b z) -> b z", b=2)[
                    bass.ds(peer, 1), :]
                ktv = blk[:, 0:KT_ELEMS].rearrange(
                    "o (p a t) -> o p a t", p=128, a=DT)
                vv = blk[:, KT_ELEMS:].rearrange(
                    "o (p h k e) -> o p h k e", p=128, h=H, k=4)

                with tc.tile_pool(name="pattn", bufs=1, space="PSUM") as pat:
                    def attn_pass(t, kA, kB, vA, vB, mask, is_local):
                        """One causal pass for head pair (2t, 2t+1).

                        kA/kB: [64,128]-sliceable K^T aps per k-tile j;
                        vA/vB: [128,64] V aps per k-tile j."""
                        poA = pat.tile([128, 512], F32, name="poA", tag="poA")
                        poB = pat.tile([128, 512], F32, name="poB", tag="poB")
                        dent = pat.tile([128, 512], F32, name="dent",
                                        tag="dent")
                        for j in range(4):
                            off = 128 * j
                            psA = pat.tile([128, 512], F32, name="psA",
                                           tag="psA")
                            psB = pat.tile([128, 512], F32, name="psB",
                                           tag="psB")
                            nc.tensor.matmul(psA[:, off:], kA(j),
                                             qt_sb[0:64, t, off:],
                                             start=True, stop=True)
                            nc.tensor.matmul(psB[:, off:], kB(j),
                                             qt_sb[64:128, t, off:],
                                             start=True, stop=True)
                            stA = stp.tile([128, 512], BF16, name="stA",
                                           tag="stA")
                            nc.scalar.activation(
                                stA[:, off:], psA[:, off:],
                                mybir.ActivationFunctionType.Exp,
                                scale=INV_SQRT_C)
                            nc.vector.tensor_mul(stA[:, off:off + 128],
                                                 stA[:, off:off + 128], mask)
                            stB = stp.tile([128, 512], BF16, name="stB",
                                           tag="stB")
                            nc.scalar.activation(
                                stB[:, off:], psB[:, off:],
                                mybir.ActivationFunctionType.Exp,
                                scale=INV_SQRT_C)
                            nc.gpsimd.tensor_mul(stB[:, off:off + 128],
                                                 stB[:, off:off + 128], mask)
                            st, sp = (j == 0), (j == 3)
                            nc.tensor.matmul(poA[0:64, off:], vA(j),
                                             stA[:, off:], start=st, stop=sp)
                            nc.tensor.matmul(dent[64:65, off:], ones_bf[:],
                                             stA[:, off:], start=st, stop=sp,
                                             tile_position=(0, 64))
                            nc.tensor.matmul(poB[64:128, off:], vB(j),
                                             stB[:, off:], start=st, stop=sp)
                            nc.tensor.matmul(dent[0:1, off:], ones_bf[:],
                                             stB[:, off:], start=st, stop=sp)
                        return poA, poB, dent

                    for t in range(DT):
                        # local pass (kt_sb/v_sb; diag mask = tril)
                        poA, poB, dent = attn_pass(
                            t,
                            lambda j: kt_sb[0:64, t, j * 128:(j + 1) * 128],
                            lambda j: kt_sb[64:128, t, j * 128:(j + 1) * 128],
                            lambda j: v_sb[:, 2 * t, j, :],
                            lambda j: v_sb[:, 2 * t + 1, j, :],
                            mask_t[:, 0], True)
                        nc.vector.tensor_copy(locA[0:64, t], poA[0:64])
                        nc.vector.tensor_copy(locA[64:65, t], dent[64:65])
                        nc.scalar.copy(locB[64:128, t], poB[64:128])
                        nc.scalar.copy(locB[0:1, t], dent[0:1])

                    for t in range(DT):
                        ktR = rem.tile([128, 512], BF16, name="ktR", tag="ktR")
                        nc.gpsimd.dma_start(ktR[:], ktv[:, :, t, :])
                        vR = rem.tile([128, 2, 4, 64], BF16, name="vR",
                                      tag="vR")
                        nc.gpsimd.dma_start(vR[:], vv[:, :, 2 * t:2 * t + 2])
                        poA, poB, dent = attn_pass(
                            t,
                            lambda j: ktR[0:64, j * 128:(j + 1) * 128],
                            lambda j: ktR[64:128, j * 128:(j + 1) * 128],
                            lambda j: vR[:, 0, j, :],
                            lambda j: vR[:, 1, j, :],
                            mask_t[:, 1], False)
                        # combine + normalize, head A (DVE) / head B (Pool)
                        sumA = sump.tile([128, 512], F32, name="sumA",
                                         tag="sumA")
                        nc.vector.tensor_add(sumA[0:64], poA[0:64],
                                             locA[0:64, t])
                        nc.vector.tensor_add(sumA[64:65], dent[64:65],
                                             locA[64:65, t])
                        nc.vector.reciprocal(sumA[64:65], sumA[64:65])
                        pbcA = pat.tile([128, 512], F32, name="pbcA",
                                        tag="pbcA")
                        nc.tensor.matmul(pbcA[0:64], onesf2[64:65, 0:64],
                                         sumA[64:65], start=True, stop=True)
                        nc.vector.tensor_mul(oT_all[0:64, t], sumA[0:64],
                                             pbcA[0:64])
                        sumB = sump.tile([128, 512], F32, name="sumB",
                                         tag="sumB")
                        nc.vector.tensor_add(sumB[64:128], poB[64:128],
                                             locB[64:128, t])
                        nc.vector.tensor_add(sumB[0:1], dent[0:1],
                                             locB[0:1, t])
                        nc.vector.reciprocal(sumB[0:1], sumB[0:1])
                        pbcB = pat.tile([128, 512], F32, name="pbcB",
                                        tag="pbcB")
                        nc.tensor.matmul(pbcB[64:128], onesf2[0:1, 0:64],
                                         sumB[0:1], start=True, stop=True)
                        nc.vector.tensor_mul(oT_all[64:128, t], sumB[64:128],
                                             pbcB[64:128])

                # ---- Wo projection + bo + residual
                bo_t = small.tile([128, DT], F32, name="bo_t", tag="bo")
                nc.sync.dma_start(bo_t[:], bo_p[l])
                x2 = xpool.tile([128, DT, 512], F32, name="x2", tag="x")
                with tc.tile_pool(name="pwo", bufs=3, space="PSUM") as pwo:
                    for half in range(2):
                        wo_t = wpool.tile([128, 4, DT, 128], BF16, name="wo_t",
                                          tag="wbig")
                        nc.sync.dma_start(wo_t[:],
                                          wop[l, :, half * 4:(half + 1) * 4])
                        for di in range(4):
                            dto = half * 4 + di
                            pw = pwo.tile([128, 512], F32, name="pw", tag="pw")
                            for et in range(DT):
                                nc.tensor.matmul(pw[:], wo_t[:, di, et],
                                                 oT_all[:, et],
                                                 start=(et == 0),
                                                 stop=(et == DT - 1))
                            nc.vector.scalar_tensor_tensor(
                                x2[:, dto], pw[:], bo_t[:, dto:dto + 1],
                                n1[:, dto], mybir.AluOpType.add,
                                mybir.AluOpType.add)

                n2 = layernorm(x2, ln2s[l], ln2b[l])

                # ---- MLP
                b1_t = small.tile([128, 32], F32, name="b1_t", tag="b1")
                nc.sync.dma_start(b1_t[:], b1_p[l])
                b2_t = small.tile([128, DT], F32, name="b2_t", tag="b2")
                nc.sync.dma_start(b2_t[:], b2_p[l])
                hT = proj.tile([128, 32, 512], BF16, name="hT", tag="vh")
                x3 = xpool.tile([128, DT, 512], F32, name="x3", tag="x")
                with tc.tile_pool(name="pmlp", bufs=2, space="PSUM") as pmlp:
                    for hp in range(8):
                        w1_t = wpool.tile([128, 4, DT, 128], BF16, name="w1_t",
                                          tag="w1")
                        nc.sync.dma_start(w1_t[:],
                                          w1p[l, :, hp * 4:(hp + 1) * 4])
                        for hi in range(4):
                            ht = hp * 4 + hi
                            ph = pmlp.tile([128, 512], F32, name="ph", tag="ph")
                            for dt_i in range(DT):
                                nc.tensor.matmul(ph[:], w1_t[:, hi, dt_i],
                                                 n2[:, dt_i],
                                                 start=(dt_i == 0),
                                                 stop=(dt_i == DT - 1))
                            nc.scalar.activation(
                                hT[:, ht], ph[:],
                                mybir.ActivationFunctionType.Relu,
                                bias=b1_t[:, ht:ht + 1])
                    for dt_i in range(DT):
                        w2_t = wpool.tile([128, 1, 32, 128], BF16, name="w2_t",
                                          tag="w2", bufs=2)
                        nc.sync.dma_start(w2_t[:], w2p[l, :, dt_i:dt_i + 1])
                        py = pmlp.tile([128, 512], F32, name="py", tag="py")
                        for ht in range(32):
                            nc.tensor.matmul(py[:], w2_t[:, 0, ht], hT[:, ht],
                                             start=(ht == 0), stop=(ht == 31))
                        nc.vector.scalar_tensor_tensor(
                            x3[:, dt_i], py[:], b2_t[:, dt_i:dt_i + 1],
                            n2[:, dt_i], mybir.AluOpType.add,
                            mybir.AluOpType.add)
                xT = x3

        # ---- final LN + LM head
        nf = layernorm(xT, None, None)
        out_r = out.rearrange("(q p) v -> p q v", p=128)
        with tc.tile_pool(name="lmw", bufs=3) as lmp, \
             tc.tile_pool(name="lmb", bufs=4) as lbp, \
             tc.tile_pool(name="osb", bufs=3) as osb, \
             tc.tile_pool(name="plm", bufs=8, space="PSUM") as plm:
            for vg in range(NVG):
                lw = lmp.tile([128, DT, 512], BF16, name="lw", tag="lw")
                nc.sync.dma_start(lw[:], lmwp[vg])
                lb = lbp.tile([1, 512], BF16, name="lb", tag="lb")
                nc.scalar.dma_start(lb[:], lmb_r[vg])
                pls = [plm.tile([128, 512], F32, name="plm", tag="plm")
                       for _ in range(4)]
                for dt_i in range(DT):
                    for qi in range(4):
                        nc.tensor.matmul(
                            pls[qi][:],
                            nf[:, dt_i, qi * 128:(qi + 1) * 128],
                            lw[:, dt_i], start=(dt_i == 0), stop=False)
                for qi in range(4):
                    nc.tensor.matmul(pls[qi][:], ones_bf1[:], lb[:],
                                     start=False, stop=True)
                ob = osb.tile([128, 4, 512], BF16, name="ob", tag="ob")
                for qi in range(4):
                    if qi < 2:
                        nc.scalar.copy(ob[:, qi], pls[qi][:])
                    else:
                        nc.vector.tensor_copy(ob[:, qi], pls[qi][:])
                nc.scalar.dma_start(out_r[:, :, vg * 512:(vg + 1) * 512],
                                    ob[:])

    nc.compile()
    return nc


def host_prep(inputs, num_layers=L):
    """Per-core in_maps + reassembly metadata from full inputs."""
    f32 = np.float32
    bf = ml_dtypes.bfloat16
    idx = np.asarray(inputs["idx"])
    tok_emb = np.asarray(inputs["tok_emb"], f32)
    pos_emb = np.asarray(inputs["pos_emb"], f32)

    def perD(a):  # [L?, D] -> [L?, 128, DT]
        a = np.asarray(a, f32)
        if a.ndim == 1:
            return np.ascontiguousarray(a.reshape(DT, 128).T)
        return np.ascontiguousarray(
            a.reshape(a.shape[0], -1, 128).transpose(0, 2, 1))

    NLx = num_layers

    def panelK(w):  # [L, D, D] -> [L, 128(p), 8(ft), 8(dt), 128(f)]
        w = np.asarray(w, f32)[:NLx].astype(bf)
        return np.ascontiguousarray(
            w.reshape(NLx, DT, 128, DT, 128).transpose(0, 2, 3, 1, 4))

    wqp = panelK(inputs["Wq"])
    wkp = panelK(inputs["Wk"])
    wop = panelK(inputs["Wo"])
    wv = np.asarray(inputs["Wv"], f32)[:NLx].astype(bf)
    wvp = np.ascontiguousarray(
        wv.reshape(NLx, DT, 128, 2, 512).transpose(0, 2, 3, 1, 4))
    w1 = np.asarray(inputs["W1"], f32)[:NLx].astype(bf)
    w1p = np.ascontiguousarray(
        w1.reshape(NLx, DT, 128, 32, 128).transpose(0, 2, 3, 1, 4))
    w2 = np.asarray(inputs["W2"], f32)[:NLx].astype(bf)
    w2p = np.ascontiguousarray(
        w2.reshape(NLx, 32, 128, DT, 128).transpose(0, 2, 3, 1, 4))

    ln1s = perD(inputs["ln1_s"])[:NLx]
    ln1b = perD(inputs["ln1_b"])[:NLx]
    ln2s = perD(inputs["ln2_s"])[:NLx]
    ln2b = perD(inputs["ln2_b"])[:NLx]
    bo_p = perD(inputs["bo"])[:NLx]
    b1_p = perD(inputs["b1"])[:NLx]
    b2_p = perD(inputs["b2"])[:NLx]
    lnfs = perD(inputs["lnf_s"])
    lnfb = perD(inputs["lnf_b"])

    lmw = np.zeros((D, VPAD), f32)
    lmw[:, :V] = np.asarray(inputs["lm_W"], f32)
    lmw = lmw.astype(bf)
    # [D, VPAD] -> [99(vg), 128(p), 8(dt), 512(c)]
    lmwp = np.ascontiguousarray(
        lmw.reshape(DT, 128, NVG, 512).transpose(2, 1, 0, 3))
    lmb = np.zeros((VPAD,), f32)
    lmb[:V] = np.asarray(inputs["lm_b"], f32)
    lmb_r = np.ascontiguousarray(lmb.reshape(NVG, 1, 512)).astype(bf)

    tri = np.tril(np.ones((128, 128), f32)).T  # mask[k, q] = 1 if k <= q
    m_ones = np.ones((128, 128), f32)
    m_zero = np.zeros((128, 128), f32)

    in_maps = []
    tiles_by_parity = []
    for c in range(8):
        b, p = c // 2, c % 2
        g_tiles = [2 * j + 1 - p for j in range(QT)]
        tiles_by_parity.append(g_tiles)
        rows = np.concatenate(
            [np.arange(g * 128, (g + 1) * 128) for g in g_tiles])
        x0 = tok_emb[idx[b, rows]] + pos_emb[rows]          # [512, D]
        x0T = np.ascontiguousarray(
            x0.T.reshape(DT, 128, 512).transpose(1, 0, 2)).astype(f32)
        masks = np.stack([tri, m_ones if p == 0 else m_zero])
        in_maps.append(dict(
            x0T=x0T, wqp=wqp, wkp=wkp, wvp=wvp, wop=wop, w1p=w1p, w2p=w2p,
            ln1s=ln1s, ln1b=ln1b, ln2s=ln2s, ln2b=ln2b,
            bo_p=bo_p, b1_p=b1_p, b2_p=b2_p, lnfs=lnfs, lnfb=lnfb,
            lmwp=lmwp, lmb_r=lmb_r, masks=masks.astype(bf),
            peer_i=np.array([[1 - p]], np.int32),
        ))
    return in_maps, tiles_by_parity


def assemble(results, tiles_by_parity):
    out = np.empty((B, T, V), np.float32)
    for c in range(8):
        b = c // 2
        co = np.asarray(results[c]["out"], dtype=np.float32)
        for j, g in enumerate(tiles_by_parity[c]):
            out[b, g * 128:(g + 1) * 128] = co[j * 128:(j + 1) * 128, :V]
    return out


_CACHE = {}


def run(inputs, num_layers=L, trace=False):
    in_maps, tiles = host_prep(inputs, num_layers)
    key = num_layers
    if key not in _CACHE:
        _CACHE[key] = build_nc(num_layers)
    nc = _CACHE[key]
    res = run_bass_kernel_spmd(nc, in_maps, core_ids=list(range(8)),
                               trace=trace)
    return assemble(res.results, tiles), res


def kernel(**inputs):
    out, _ = run(inputs, L)
    return out
